# revision 1
# baseline (speedup 1.0000x reference)
"""Trainium2 Bass kernel for nn_AttributeAttn (dense_transformer, memory-bound).

Math (collapsed reference):
    u = W.T @ v; uh, ue = u[:H], u[H:]
    hv[n,b] = hidden[n,b,:] @ uh          # the big 128MB reduction
    ev[c,b] = enc[c,b,:] @ ue
    bias    = b @ v
    out[b,n,c] = softmax_c(tanh(hv[n,b] + ev[c,b] + bias))

Distribution: data-parallel over B (4 batches per core, 8 cores).
Per-core device program:
  - load vb (v,b columns), W (replicated), enc shard; compute u row via
    streaming f32r matmuls; transpose u to column form via rank-1 matmuls;
    compute ev row + bias; broadcast ev+bias to all partitions via a
    contract-1 matmul.
  - stream the 16MB hidden shard (host-pretransposed to [H, N*b], cast to
    f32r during DMA); contract over H with TensorE (f32r, 1 cycle/row);
    per 128-row block: transpose hv to partition form, add ev+bias, tanh,
    exp, row-sum, reciprocal, scale, DMA out.

Host side only shards/transposes/concatenates (no module math on host).
"""
import sys
import types

import numpy as np

# The container's antenv stub lacks axon_hooks; provide it so trace=True
# works when the test harness requests profiling. Harmless otherwise.
if "antenv.axon_hooks" not in sys.modules:
    _hooks_mod = types.ModuleType("antenv.axon_hooks")
    try:
        from trn_agent_boot.trn_boot import _ntff_profile_via_ctypes
        _ntff_hook = _ntff_profile_via_ctypes("/opt/axon/libaxon_pjrt.so")
    except Exception:
        _ntff_hook = None
    _hooks_mod.get_axon_ntff_profile_hook = lambda: _ntff_hook
    _hooks_mod.set_axon_ntff_profile_hook = lambda h: None
    sys.modules["antenv.axon_hooks"] = _hooks_mod

import concourse.bacc as bacc
import concourse.tile as tile
from concourse import mybir
from concourse.bass_utils import run_bass_kernel_spmd

f32 = mybir.dt.float32
f32r = mybir.dt.float32r
AF = mybir.ActivationFunctionType
X = mybir.AxisListType.X
ADD = mybir.AluOpType.add

N, B, H = 1024, 32, 1024
C, K = 64, 512
NCORES = 8
BPC = B // NCORES            # 4 batches per core
NB = N * BPC                 # 4096 free elements of the hv contraction
HC = H // 128                # 8 h-chunks
KC = K // 128                # 4 k-chunks
JC = (H + K) // 128          # 12 u columns
NBLK = N // 128              # 8 n-blocks per core
FW = BPC * C                 # 256 free (bb, c) elements per n-block

# Hidden streams in four spans of 2 n-blocks. Small spans keep the PE's
# chunk-arrival gaps under the ~3.4us HAM re-throttle window (so matmuls run
# at 2.4 GHz) and let the softmax/store pipeline run throughout the stream.
# Every (h-chunk, span) tile gets its own SBUF slot so the DMA stream never
# backpressures.
SPANS = [(k, k + 2) for k in range(0, NBLK, 2)]

# Set by test harness to capture an NTFF profile.
TRACE = False
TRACE_KW = {}
LAST_RESULT = None

_cached = None


def _build():
    nc = bacc.Bacc(None, target_bir_lowering=False)
    hid_d = nc.dram_tensor("hid", [H, NB], f32, kind="ExternalInput")
    enc_d = nc.dram_tensor("enc", [K, FW], f32, kind="ExternalInput")
    w_d = nc.dram_tensor("w", [H, H + K], f32, kind="ExternalInput")
    vb_d = nc.dram_tensor("vb", [128, 2 * HC], f32, kind="ExternalInput")
    out_d = nc.dram_tensor("out", [N, FW], f32, kind="ExternalOutput")

    with tile.TileContext(nc) as tc:
        with (
            tc.tile_pool(name="consts", bufs=1) as consts,
            tc.tile_pool(name="wpool", bufs=2) as wpool,
            tc.tile_pool(name="hpool", bufs=1) as hpool,
            tc.tile_pool(name="work", bufs=3) as work,
        ):
            # --- constants (all loads on the sync HWDGE ring, f32r bitcast:
            # the PE's f32r path reads 4-byte values; no cast pass needed) ---
            vb_sb = consts.tile([128, 2 * HC], f32r, tag="vb")
            nc.sync.dma_start(out=vb_sb, in_=vb_d.bitcast(f32r)[:, :])
            ones_sb = consts.tile([1, 128], f32, tag="ones")
            nc.vector.memset(ones_sb, 1.0)
            ones_r = consts.tile([1, 128], f32r, tag="ones_r")
            nc.vector.tensor_copy(ones_r, ones_sb)
            enc_sb = consts.tile([128, KC, FW], f32r, tag="enc")
            nc.sync.dma_start(
                out=enc_sb,
                in_=enc_d.bitcast(f32r).rearrange("(kc p) f -> p kc f", p=128))

            # --- W stream (before hid so u is ready early). DMA issue on a
            # HWDGE ring costs ~0.65us serialized per dma_start, so split
            # every stream across both rings (sync + scalar). ---
            rings = [nc.sync, nc.scalar]
            w_tiles = []
            for ic in range(HC):
                # 4-deep W pipeline: consumption (3 fast matmuls per chunk)
                # never gates the stream now that warmup runs on enc
                w_sb = wpool.tile([128, H + K], f32r, tag="w", bufs=4,
                                  name=f"w_{ic}")
                rings[ic % 2].dma_start(
                    out=w_sb,
                    in_=w_d.bitcast(f32r)[ic * 128:(ic + 1) * 128, :])
                w_tiles.append(w_sb)


            with tc.tile_pool(name="ps_setup", bufs=1, space="PSUM") as pset:
                # PE clock warmup: the HAM gate keeps the PE at 1.2 GHz until
                # it has been busy ~3.4us continuously. Burn that in on the
                # early-arriving enc tile (so nothing downstream waits on the
                # warmup), then sprinkle keep-warm matmuls through the W
                # phase so the clock never drops back.
                warm_ps = pset.tile([1, FW], f32, tag="warm")

                def keep_warm(n, tgt=None):
                    for _ in range(n):
                        nc.tensor.matmul(
                            tgt if tgt is not None else warm_ps,
                            vb_sb[:, 0:1], enc_sb[:, 0, :],
                            start=True, stop=True)

                keep_warm(16)

                # u row = v.T @ W  (1, 1536)
                u_ps = pset.tile([1, 3, 512], f32, tag="u")
                for ic in range(HC):
                    for jb in range(3):
                        nc.tensor.matmul(
                            u_ps[:, jb, :],
                            vb_sb[:, ic:ic + 1],
                            w_tiles[ic][:, jb * 512:(jb + 1) * 512],
                            start=(ic == 0), stop=(ic == HC - 1))
                    keep_warm(3)
                u_row = consts.tile([1, JC, 128], f32, tag="urow")
                nc.vector.tensor_copy(
                    u_row.rearrange("p a b -> p (a b)").rearrange(
                        "p (x y) -> p x y", x=3), u_ps)

                # u columns (128, 12) via rank-1 transposes
                uc_ps = pset.tile([128, JC], f32, tag="uc")
                for jc in range(JC):
                    nc.tensor.matmul(
                        uc_ps[:, jc:jc + 1], u_row[0:1, jc, :],
                        ones_sb[:, 0:1], start=True, stop=True)
                ucols = consts.tile([128, JC], f32r, tag="ucols")
                nc.vector.tensor_copy(ucols, uc_ps)

                # bias = b @ v
                bias_ps = pset.tile([1, 1], f32, tag="bias")
                for ic in range(HC):
                    nc.tensor.matmul(
                        bias_ps, vb_sb.bitcast(f32)[:, ic:ic + 1],
                        vb_sb.bitcast(f32)[:, HC + ic:HC + ic + 1],
                        start=(ic == 0), stop=(ic == HC - 1))
                bias_sb = consts.tile([1, 1], f32, tag="bias_sb")
                nc.vector.tensor_copy(bias_sb, bias_ps)

                # ev row (1, 256) then evb = ev + bias
                ev_ps = pset.tile([1, FW], f32, tag="ev")
                for kc in range(KC):
                    nc.tensor.matmul(
                        ev_ps, ucols[:, HC + kc:HC + kc + 1],
                        enc_sb[:, kc, :], start=(kc == 0), stop=(kc == KC - 1))
                evb_row = consts.tile([1, FW], f32r, tag="evb")
                nc.vector.tensor_scalar_add(evb_row, ev_ps, bias_sb[:, 0:1])
                # ev+bias broadcast to all partitions: ones (x) evb_row
                bc_ps = pset.tile([128, FW], f32, tag="bc")
                nc.tensor.matmul(bc_ps, ones_r, evb_row, start=True,
                                 stop=True)
                evb_rep = consts.tile([128, FW], f32, tag="evbrep")
                nc.vector.tensor_copy(evb_rep, bc_ps)
                # bridge the gap between the u phase and the first hidden
                # chunks so the PE clock stays at 2.4 GHz
                keep_warm(20)

            # --- hidden stream: (hc, span) tiles, span-major ---
            hid_sb = {}
            for s, (k0, k1) in enumerate(SPANS):
                for hc in range(HC):
                    t = hpool.tile([128, (k1 - k0) * 512], f32r,
                                   tag=f"hid{hc}", bufs=3,
                                   name=f"hid_{hc}_{s}")
                    rings[hc % 2].dma_start(
                        out=t,
                        in_=hid_d.bitcast(f32r)[
                            hc * 128:(hc + 1) * 128, k0 * 512:k1 * 512])
                    hid_sb[hc, s] = t

            # --- main: hv contraction + softmax, per 128-row n-block.
            # The score tile is built directly in PSUM by TensorE:
            #   score = ones (x) (ev+bias)  +  sum_bb hv_row_bb (x) ones_C
            # so no hv transpose or partition-broadcast pass is needed. ---
            with (
                tc.tile_pool(name="dram", bufs=1, space="DRAM") as dpool,
                tc.tile_pool(name="ps_main", bufs=1, space="PSUM") as pp,
            ):
                bounce = dpool.tile([NBLK, 512], f32, tag="bounce")
                rows = {}

                def contract_span(s, pending):
                    k0, k1 = SPANS[s]
                    accs = {k: pp.tile([1, 512], f32, tag="acc", bufs=4,
                                       name=f"acc_{k}")
                            for k in range(k0, k1)}
                    for hc in range(HC):
                        for k in range(k0, k1):
                            kl = k - k0
                            nc.tensor.matmul(
                                accs[k], ucols[:, hc:hc + 1],
                                hid_sb[hc, s][:, kl * 512:(kl + 1) * 512],
                                start=(hc == 0), stop=(hc == HC - 1))
                        # previous span's finish work slots between chunk
                        # batches instead of queueing behind this span's
                        # last matmul
                        if hc in (0, 3) and pending:
                            finish(pending.pop(0))
                    for k in range(k0, k1):
                        row = work.tile([1, 512], f32, tag="row",
                                        name=f"row_{k}", bufs=NBLK)
                        nc.vector.tensor_copy(row, accs[k])
                        rows[k] = row

                def finish(k, tail=False):
                    sc = work.tile([128, FW], f32, tag="sc")
                    if tail:
                        # PE is idle once the stream ends: build the score
                        # directly in PSUM (shortest-latency chain)
                        rowv = rows[k].rearrange("p (n bb) -> p bb n",
                                                 bb=BPC)
                        sc_ps = pp.tile([128, FW], f32, tag="score", bufs=2,
                                        name=f"score_{k}")
                        nc.tensor.matmul(
                            sc_ps, ones_r, evb_row, start=True, stop=False)
                        for bb in range(BPC):
                            nc.tensor.matmul(
                                sc_ps[:, bb * C:(bb + 1) * C],
                                rowv[0:1, bb, :], ones_sb[:, 0:C],
                                start=False, stop=(bb == BPC - 1),
                                skip_group_check=True)
                        nc.scalar.activation(out=sc, in_=sc_ps, func=AF.Tanh)
                    else:
                        # mid-stream: transpose the hv row to partition form
                        # via a DRAM bounce on the idle SWDGE path — zero PE
                        nc.gpsimd.dma_start(out=bounce[k:k + 1, :],
                                            in_=rows[k])
                        hvt = work.tile([128, BPC], f32, tag="hvt", bufs=3,
                                        name=f"hvt_{k}")
                        nc.gpsimd.dma_start(
                            out=hvt,
                            in_=bounce[k:k + 1, :].rearrange(
                                "o (p bb) -> (o p) bb", p=128))
                        for bb in range(BPC):
                            nc.vector.tensor_scalar_add(
                                sc[:, bb * C:(bb + 1) * C],
                                evb_rep[:, bb * C:(bb + 1) * C],
                                hvt[:, bb:bb + 1])
                        nc.scalar.activation(out=sc, in_=sc, func=AF.Tanh)
                    nc.scalar.activation(out=sc, in_=sc, func=AF.Exp)
                    den = work.tile([128, BPC], f32, tag="den")
                    nc.vector.tensor_reduce(
                        den, sc.rearrange("p (bb c) -> p bb c", c=C),
                        axis=X, op=ADD)
                    nc.vector.reciprocal(den, den)
                    o_sb = work.tile([128, FW], f32, tag="o")
                    for bb in range(BPC):
                        nc.vector.tensor_scalar_mul(
                            o_sb[:, bb * C:(bb + 1) * C],
                            sc[:, bb * C:(bb + 1) * C],
                            den[:, bb:bb + 1])
                    if tail:
                        # sync ring is idle at the tail and has the lowest
                        # store latency
                        nc.sync.dma_start(
                            out=out_d[k * 128:(k + 1) * 128, :], in_=o_sb)
                    else:
                        nc.gpsimd.dma_start(
                            out=out_d[k * 128:(k + 1) * 128, :], in_=o_sb)

                # finishes are skewed one span behind the contraction and
                # spliced between the next span's chunk batches
                pending = []
                for s in range(len(SPANS)):
                    contract_span(s, pending)
                    pending.extend(range(*SPANS[s]))
                for k in pending:
                    finish(k, tail=True)
    nc.compile()
    return nc


def kernel(**inputs):
    global _cached, LAST_RESULT
    hidden = np.ascontiguousarray(inputs["hidden"], dtype=np.float32)
    enc = np.ascontiguousarray(inputs["encoder_outputs"], dtype=np.float32)
    W = np.ascontiguousarray(inputs["W"], dtype=np.float32)
    b = np.ascontiguousarray(inputs["b"], dtype=np.float32)
    v = np.ascontiguousarray(inputs["v"], dtype=np.float32)

    if _cached is None:
        _cached = _build()
    nc = _cached

    # vb: column ic holds v[ic*128:(ic+1)*128]; column HC+ic holds b chunk.
    vb = np.concatenate(
        [v.reshape(HC, 128).T, b.reshape(HC, 128).T], axis=1)
    vb = np.ascontiguousarray(vb, dtype=np.float32)

    in_maps = []
    for j in range(NCORES):
        bsl = slice(j * BPC, (j + 1) * BPC)
        # (H, N, bpc) -> (H, N*bpc); free index = n*BPC + bb
        hid_t = np.ascontiguousarray(
            np.transpose(hidden[:, bsl, :], (2, 0, 1)).reshape(H, NB))
        # (K, bpc, C) -> (K, bpc*C); free index = bb*C + c
        enc_t = np.ascontiguousarray(
            np.transpose(enc[:, bsl, :], (2, 1, 0)).reshape(K, FW))
        in_maps.append({"hid": hid_t, "enc": enc_t, "w": W, "vb": vb})

    res = run_bass_kernel_spmd(
        nc, in_maps, core_ids=list(range(NCORES)), trace=TRACE, **TRACE_KW)
    LAST_RESULT = res

    out = np.empty((B, N, C), dtype=np.float32)
    for j in range(NCORES):
        o = res.results[j]["out"].reshape(N, BPC, C)
        out[j * BPC:(j + 1) * BPC] = o.transpose(1, 0, 2)
    return out



# revision 3
# speedup vs baseline: 1.6436x; 1.6436x over previous
"""Trainium2 Bass kernel for nn_AttributeAttn (dense_transformer, memory-bound).

Math (collapsed reference):
    u = W.T @ v; uh, ue = u[:H], u[H:]
    hv[n,b] = hidden[n,b,:] @ uh          # the big reduction
    ev[c,b] = enc[c,b,:] @ ue
    bias    = b @ v
    out[b,n,c] = softmax_c(tanh(hv[n,b] + ev[c,b] + bias))

Distribution: data-parallel over B (4 batches per core, 8 cores).

The problem is pure HBM streaming: per core the inputs are hidden 8MB,
W 3MB, enc 0.25MB (all bf16; the 2e-2 rel-err gate leaves ~100x slack
vs f32, and bf16 keeps the score error ~1e-3), output 0.5MB bf16
(host upcasts). Every load is a single fully-contiguous >=1MB DMA
(small/strided descriptors cap HBM at ~70%; contiguous >=1MB transfers
reach ~340-425 GB/s of the 358 GB/s per-core HBM limit).

Device program per core:
  - two HWDGE rings stream concurrently: sync gets [vb+W_lo, hid0,2,4,6],
    scalar gets [W_hi+enc, hid1,3,5,7]; hidden is host-packed n-block-major
    so each 1MB tile holds all 8 h-chunks of one 128-row n-block.
  - u row via 24 streaming bf16 matmuls; u columns via rank-1 transposes;
    ev row + bias; evb = ev + bias.
  - per n-block: contract over H in PSUM (8 matmuls), then build the score
    tile directly in PSUM with TensorE (ones (x) evb + rank-1 hv
    broadcast), tanh, exp, row-sum, reciprocal, scale, bf16 store.

Host side only shards/transposes/casts (no module math on host).
"""
import sys
import types

import numpy as np
import ml_dtypes

BF = ml_dtypes.bfloat16

# The container's antenv stub lacks axon_hooks; provide it so trace=True
# works when the test harness requests profiling. Harmless otherwise.
if "antenv.axon_hooks" not in sys.modules:
    _hooks_mod = types.ModuleType("antenv.axon_hooks")
    try:
        from trn_agent_boot.trn_boot import _ntff_profile_via_ctypes
        _ntff_hook = _ntff_profile_via_ctypes("/opt/axon/libaxon_pjrt.so")
    except Exception:
        _ntff_hook = None
    _hooks_mod.get_axon_ntff_profile_hook = lambda: _ntff_hook
    _hooks_mod.set_axon_ntff_profile_hook = lambda h: None
    sys.modules["antenv.axon_hooks"] = _hooks_mod

import concourse.bacc as bacc
import concourse.tile as tile
from concourse import mybir
from concourse.bass_utils import run_bass_kernel_spmd

f32 = mybir.dt.float32
bf16 = mybir.dt.bfloat16
AF = mybir.ActivationFunctionType
X = mybir.AxisListType.X
ADD = mybir.AluOpType.add

N, B, H = 1024, 32, 1024
C, K = 64, 512
NCORES = 8
BPC = B // NCORES            # 4 batches per core
NB = N * BPC                 # 4096 free elements of the hv contraction
HC = H // 128                # 8 h-chunks
KC = K // 128                # 4 k-chunks
JC = (H + K) // 128          # 12 u columns
NBLK = N // 128              # 8 n-blocks per core
FW = BPC * C                 # 256 free (bb, c) elements per n-block
BW = 128 * BPC               # 512 hv free elements per n-block
WROW = H + K                 # 1536
WHALF = (HC // 2) * WROW     # 6144 cols of one W half per partition

# Set by test harness to capture an NTFF profile.
TRACE = False
TRACE_KW = {}
LAST_RESULT = None

_cached = None


def _build():
    nc = bacc.Bacc(None, target_bir_lowering=False)
    # wlo packs vb (v,b chunk columns) ahead of W h-chunks 0..3; whi has 4..7.
    wlo_d = nc.dram_tensor("wlo", [128, 2 * HC + WHALF], bf16, kind="ExternalInput")
    whi_d = nc.dram_tensor("whi", [128, WHALF + KC * FW], bf16, kind="ExternalInput")
    hid_d = nc.dram_tensor("hid", [128, NBLK * HC * BW], bf16, kind="ExternalInput")
    out_d = nc.dram_tensor("out", [N, FW], bf16, kind="ExternalOutput")

    with tile.TileContext(nc) as tc:
        with (
            tc.tile_pool(name="consts", bufs=1) as consts,
            tc.tile_pool(name="work", bufs=3) as work,
        ):
            # --- loads: both HWDGE rings stream concurrently (the SDMA
            # engines round-robin between the two ring rows at packet
            # granularity), so W halves land in ~9us while hidden streams.
            wlo_sb = consts.tile([128, 2 * HC + WHALF], bf16, tag="wlo")
            nc.sync.dma_start(out=wlo_sb, in_=wlo_d[:, :])
            whi_sb = consts.tile([128, WHALF + KC * FW], bf16, tag="whi")
            nc.scalar.dma_start(out=whi_sb, in_=whi_d[:, :])

            vb_sb = wlo_sb[:, 0:2 * HC]
            enc_sb = whi_sb[:, WHALF:]

            def wchunk(ic, j0, j1):
                if ic < HC // 2:
                    return wlo_sb[:, 2 * HC + ic * WROW + j0:2 * HC + ic * WROW + j1]
                ic -= HC // 2
                return whi_sb[:, ic * WROW + j0:ic * WROW + j1]

            hid_sb = []
            rings = [nc.sync, nc.scalar]
            for k in range(NBLK):
                t = consts.tile([128, HC * BW], bf16, tag=f"hid{k}")
                rings[k % 2].dma_start(
                    out=t, in_=hid_d[:, k * HC * BW:(k + 1) * HC * BW])
                hid_sb.append(t)

            ones_f = consts.tile([1, 128], f32, tag="ones_f")
            nc.vector.memset(ones_f, 1.0)
            ones = consts.tile([1, 128], bf16, tag="ones")
            nc.vector.tensor_copy(ones, ones_f)

            with tc.tile_pool(name="ps_setup", bufs=1, space="PSUM") as pset:
                # u row = v.T @ W  (1, 1536); 3 psum banks, 512-wide matmuls
                u_ps = pset.tile([1, 3, 512], f32, tag="u")
                for ic in range(HC):
                    for jb in range(3):
                        nc.tensor.matmul(
                            u_ps[:, jb, :],
                            vb_sb[:, ic:ic + 1],
                            wchunk(ic, jb * 512, (jb + 1) * 512),
                            start=(ic == 0), stop=(ic == HC - 1))
                u_row = consts.tile([1, JC, 128], bf16, tag="urow")
                nc.vector.tensor_copy(
                    u_row.rearrange("p a b -> p (a b)").rearrange(
                        "p (x y) -> p x y", x=3), u_ps)

                # u columns (128, 12) via rank-1 transposes
                uc_ps = pset.tile([128, JC], f32, tag="uc")
                for jc in range(JC):
                    nc.tensor.matmul(
                        uc_ps[:, jc:jc + 1], u_row[0:1, jc, :],
                        ones[:, 0:1], start=True, stop=True)
                ucols = consts.tile([128, JC], bf16, tag="ucols")
                nc.vector.tensor_copy(ucols, uc_ps)

                # bias = b @ v
                bias_ps = pset.tile([1, 1], f32, tag="bias")
                for ic in range(HC):
                    nc.tensor.matmul(
                        bias_ps, vb_sb[:, ic:ic + 1],
                        vb_sb[:, HC + ic:HC + ic + 1],
                        start=(ic == 0), stop=(ic == HC - 1))
                bias_sb = consts.tile([1, 1], f32, tag="bias_sb")
                nc.vector.tensor_copy(bias_sb, bias_ps)

                # ev row (1, 256) then evb = ev + bias
                ev_ps = pset.tile([1, FW], f32, tag="ev")
                for kc in range(KC):
                    nc.tensor.matmul(
                        ev_ps, ucols[:, HC + kc:HC + kc + 1],
                        enc_sb[:, kc * FW:(kc + 1) * FW],
                        start=(kc == 0), stop=(kc == KC - 1))
                evb_row = consts.tile([1, FW], bf16, tag="evb")
                nc.vector.tensor_scalar_add(evb_row, ev_ps, bias_sb[:, 0:1])

            # --- per n-block: contract over H, build score in PSUM, softmax
            with tc.tile_pool(name="ps_main", bufs=1, space="PSUM") as pp:
                for k in range(NBLK):
                    acc = pp.tile([1, BW], f32, tag="acc", bufs=3,
                                  name=f"acc_{k}")
                    for hc in range(HC):
                        nc.tensor.matmul(
                            acc, ucols[:, hc:hc + 1],
                            hid_sb[k][:, hc * BW:(hc + 1) * BW],
                            start=(hc == 0), stop=(hc == HC - 1))
                    row = work.tile([1, BW], bf16, tag="row", bufs=3,
                                    name=f"row_{k}")
                    nc.vector.tensor_copy(row, acc)

                    # score = ones (x) evb + sum_bb hv_row_bb (x) ones_C
                    rowv = row.rearrange("p (n bb) -> p bb n", bb=BPC)
                    sc_ps = pp.tile([128, FW], f32, tag="score", bufs=3,
                                    name=f"score_{k}")
                    nc.tensor.matmul(sc_ps, ones, evb_row,
                                     start=True, stop=False)
                    for bb in range(BPC):
                        nc.tensor.matmul(
                            sc_ps[:, bb * C:(bb + 1) * C],
                            rowv[0:1, bb, :], ones[:, 0:C],
                            start=False, stop=(bb == BPC - 1),
                            skip_group_check=True)

                    sc = work.tile([128, FW], f32, tag="sc", bufs=2,
                                   name=f"sc_{k}")
                    nc.scalar.activation(out=sc, in_=sc_ps, func=AF.Tanh)
                    nc.scalar.activation(out=sc, in_=sc, func=AF.Exp)
                    den = work.tile([128, BPC], f32, tag="den", bufs=2,
                                    name=f"den_{k}")
                    nc.vector.tensor_reduce(
                        den, sc.rearrange("p (bb c) -> p bb c", c=C),
                        axis=X, op=ADD)
                    nc.vector.reciprocal(den, den)
                    o_sb = work.tile([128, FW], bf16, tag="o", bufs=2,
                                     name=f"o_{k}")
                    for bb in range(BPC):
                        nc.vector.tensor_scalar_mul(
                            o_sb[:, bb * C:(bb + 1) * C],
                            sc[:, bb * C:(bb + 1) * C],
                            den[:, bb:bb + 1])
                    # stores ride SWDGE (separate queue row) so the HWDGE
                    # load streams never stall; the last two switch to the
                    # by-then-idle HWDGE rings for low completion latency.
                    if k < NBLK - 2:
                        store_eng = nc.gpsimd
                    else:
                        store_eng = rings[k % 2]
                    store_eng.dma_start(
                        out=out_d[k * 128:(k + 1) * 128, :], in_=o_sb)
    nc.compile()
    return nc


def kernel(**inputs):
    global _cached, LAST_RESULT
    hidden = np.asarray(inputs["hidden"], dtype=np.float32)
    enc = np.asarray(inputs["encoder_outputs"], dtype=np.float32)
    W = np.asarray(inputs["W"], dtype=np.float32)
    b = np.asarray(inputs["b"], dtype=np.float32)
    v = np.asarray(inputs["v"], dtype=np.float32)

    if _cached is None:
        _cached = _build()
    nc = _cached

    # vb: column ic holds v[ic*128:(ic+1)*128]; column HC+ic holds b chunk.
    vb = np.concatenate(
        [v.reshape(HC, 128).T, b.reshape(HC, 128).T], axis=1).astype(BF)
    # W partition-major: wt[p, ic*1536 + j] = W[ic*128 + p, j]
    wt = W.astype(BF).reshape(HC, 128, WROW).transpose(1, 0, 2)
    wlo = np.ascontiguousarray(
        np.concatenate([vb, wt[:, :HC // 2].reshape(128, WHALF)], axis=1))
    whi_w = wt[:, HC // 2:].reshape(128, WHALF)

    hb = hidden.astype(BF)
    eb = enc.astype(BF)

    in_maps = []
    for j in range(NCORES):
        bsl = slice(j * BPC, (j + 1) * BPC)
        # hid: [p, (k, hc, fn*BPC+bb)] so each n-block tile is one
        # contiguous 1MB DMA holding all 8 h-chunks of that block.
        x = hb[:, bsl, :]                                   # (N, BPC, H)
        x = x.transpose(2, 0, 1)                            # (H, N, BPC)
        x = x.reshape(HC, 128, NBLK, 128 * BPC)             # (hc, p, k, f)
        hid_t = np.ascontiguousarray(
            x.transpose(1, 2, 0, 3).reshape(128, NBLK * HC * BW))
        # enc: [p, kc*FW + bb*C + c]
        e = eb[:, bsl, :].transpose(2, 1, 0)                # (K, BPC, C)
        e = e.reshape(KC, 128, FW).transpose(1, 0, 2)
        enc_t = np.ascontiguousarray(e.reshape(128, KC * FW))
        whi = np.ascontiguousarray(np.concatenate([whi_w, enc_t], axis=1))
        in_maps.append({"hid": hid_t, "wlo": wlo, "whi": whi})

    res = run_bass_kernel_spmd(
        nc, in_maps, core_ids=list(range(NCORES)), trace=TRACE, **TRACE_KW)
    LAST_RESULT = res

    out = np.empty((B, N, C), dtype=np.float32)
    for j in range(NCORES):
        o = res.results[j]["out"].astype(np.float32).reshape(N, BPC, C)
        out[j * BPC:(j + 1) * BPC] = o.transpose(1, 0, 2)
    return out


# revision 4
# speedup vs baseline: 1.8300x; 1.1134x over previous
"""Trainium2 Bass kernel for nn_AttributeAttn (dense_transformer, memory-bound).

Math (collapsed reference):
    u = W.T @ v; uh, ue = u[:H], u[H:]
    hv[n,b] = hidden[n,b,:] @ uh          # the big reduction
    ev[c,b] = enc[c,b,:] @ ue
    bias    = b @ v
    out[b,n,c] = softmax_c(tanh(hv[n,b] + ev[c,b] + bias))

Distribution: data-parallel over B (4 batches per core, 8 cores).

The problem is pure HBM streaming: per core the inputs are hidden 8MB,
W 3MB, enc 0.25MB (all bf16; the 2e-2 rel-err gate leaves ~100x slack
vs f32 and bf16 keeps the score error ~1e-3), output 0.5MB bf16 (host
upcasts). Every load is a fully-contiguous DMA (>=0.75MB reaches
~400GB/s of the per-core HBM bandwidth; small or strided descriptors
choke at <70%).

Device schedule per core (two HWDGE rings stream concurrently; the 16
SDMA engines round-robin between the ring rows at packet granularity):
  sync  : [vb+Wq0, Wq1, hid0, hid2, hid4, hid6a, hid6b]
  scalar: [Wq2, Wq3+enc, hid1, hid3, hid5, hid7a, hid7b]
W arrives in quarters so the u matmuls overlap the W stream; ucols is
ready before hid0 lands.  The last two hidden blocks stream in 0.5MB
halves to shorten the arrival tail.  Two f32 warm-up matmuls at the
head hold the PE's HAM clock gate at 2.4 GHz (matmul cost is
moving-free-size cycles; hidden = 32K columns = 13.7us of PE that must
run warm to hide under the ~29us stream), and a dummy Tanh preloads
the ACT table set during the stream.  Per n-block: contract over H in
PSUM, build the score tile in PSUM with TensorE (ones (x) evb + rank-1
hv broadcast), tanh, exp, row-sum, reciprocal, scale into a resident
output tile; the output leaves in two 0.25MB stores (gpsimd mid-stream,
sync at the tail).

Host side only shards/transposes/casts (no module math on host).
"""
import sys
import types

import numpy as np
import ml_dtypes

BF = ml_dtypes.bfloat16

# The container's antenv stub lacks axon_hooks; provide it so trace=True
# works when the test harness requests profiling. Harmless otherwise.
if "antenv.axon_hooks" not in sys.modules:
    _hooks_mod = types.ModuleType("antenv.axon_hooks")
    try:
        from trn_agent_boot.trn_boot import _ntff_profile_via_ctypes
        _ntff_hook = _ntff_profile_via_ctypes("/opt/axon/libaxon_pjrt.so")
    except Exception:
        _ntff_hook = None
    _hooks_mod.get_axon_ntff_profile_hook = lambda: _ntff_hook
    _hooks_mod.set_axon_ntff_profile_hook = lambda h: None
    sys.modules["antenv.axon_hooks"] = _hooks_mod

import concourse.bacc as bacc
import concourse.tile as tile
from concourse import mybir
from concourse.bass_utils import run_bass_kernel_spmd

f32 = mybir.dt.float32
bf16 = mybir.dt.bfloat16
AF = mybir.ActivationFunctionType
X = mybir.AxisListType.X
ADD = mybir.AluOpType.add

N, B, H = 1024, 32, 1024
C, K = 64, 512
NCORES = 8
BPC = B // NCORES            # 4 batches per core
HC = H // 128                # 8 h-chunks
KC = K // 128                # 4 k-chunks
JC = (H + K) // 128          # 12 u columns
NBLK = N // 128              # 8 n-blocks per core
FW = BPC * C                 # 256 free (bb, c) elements per n-block
BW = 128 * BPC               # 512 hv free elements per n-block
WROW = H + K                 # 1536
WQ = 2 * WROW                # one W quarter (2 h-chunks) per partition
ENC_W = KC * FW              # 1024 enc columns per partition
HB = NBLK * HC * BW          # hid columns per partition

# Set by test harness to capture an NTFF profile.
TRACE = False
TRACE_KW = {}
LAST_RESULT = None

_cached = None


def _build():
    nc = bacc.Bacc(None, target_bir_lowering=False)
    q0_d = nc.dram_tensor("q0", [128, 2 * HC + WQ], bf16, kind="ExternalInput")
    q1_d = nc.dram_tensor("q1", [128, WQ], bf16, kind="ExternalInput")
    q2_d = nc.dram_tensor("q2", [128, WQ], bf16, kind="ExternalInput")
    q3_d = nc.dram_tensor("q3", [128, WQ + ENC_W], bf16, kind="ExternalInput")
    hid_d = nc.dram_tensor("hid", [128, HB], bf16, kind="ExternalInput")
    out_d = nc.dram_tensor("out", [128, NBLK * FW], bf16, kind="ExternalOutput")

    with tile.TileContext(nc) as tc:
        with (
            tc.tile_pool(name="consts", bufs=1) as consts,
            tc.tile_pool(name="work", bufs=3) as work,
            tc.tile_pool(name="ps_warm", bufs=1, space="PSUM") as pw,
        ):
            # --- loads ---
            q_sb = []
            rings = [nc.sync, nc.scalar]
            for qi, qd in enumerate([q0_d, q1_d, q2_d, q3_d]):
                t = consts.tile(list(qd.shape), bf16, tag=f"q{qi}")
                rings[qi // 2].dma_start(out=t, in_=qd[:, :])
                q_sb.append(t)
            vb_sb = q_sb[0][:, 0:2 * HC]
            enc_sb = q_sb[3][:, WQ:]

            def wchunk(ic, j0, j1):
                off = (2 * HC if ic < 2 else 0) + (ic % 2) * WROW
                return q_sb[ic // 2][:, off + j0:off + j1]

            hid_sb = []
            for k in range(NBLK - 2):
                t = consts.tile([128, HC * BW], bf16, tag=f"hid{k}")
                rings[k % 2].dma_start(
                    out=t, in_=hid_d[:, k * HC * BW:(k + 1) * HC * BW])
                hid_sb.append(t)
            halves = {}
            for hf in range(2):            # arrival order 6a, 7a, 6b, 7b
                for k in (6, 7):
                    t = consts.tile([128, HC * BW // 2], bf16,
                                    tag=f"hid{k}{hf}")
                    off = k * HC * BW + hf * (HC * BW // 2)
                    rings[k % 2].dma_start(
                        out=t, in_=hid_d[:, off:off + HC * BW // 2])
                    halves[k, hf] = t

            # --- PE warm-up + ACT table preload (both overlap the stream).
            # Two f32 matmuls (4 cyc/row, ~3.4us at the cold 1.2GHz clock)
            # hold the HAM activity window busy until real work arrives.
            warm_src = consts.tile([128, 512], f32, tag="warm_src")
            nc.vector.memset(warm_src, 1.0)
            ones_f = consts.tile([1, 128], f32, tag="ones_f")
            nc.vector.memset(ones_f, 1.0)
            ones = consts.tile([1, 128], bf16, tag="ones")
            nc.vector.tensor_copy(ones, ones_f)
            warm_bf = consts.tile([128, 512], bf16, tag="warm_bf")
            nc.vector.tensor_copy(warm_bf, warm_src)
            warm_ps = pw.tile([1, 512], f32, tag="warm")
            tpre = work.tile([1, 1], f32, tag="tpre")
            nc.scalar.activation(out=tpre, in_=warm_src[0:1, 0:1],
                                 func=AF.Tanh)

            def warm(n, wide=False):
                for _ in range(n):
                    if wide:
                        nc.tensor.matmul(warm_ps, warm_src[:, 0:1], warm_src,
                                         start=True, stop=True)
                    else:
                        nc.tensor.matmul(warm_ps, warm_bf[:, 0:1], warm_bf,
                                         start=True, stop=True)

            warm(2, wide=True)

            with tc.tile_pool(name="ps_setup", bufs=1, space="PSUM") as pset:
                # u row = v.T @ W (1, 1536), one W quarter at a time as the
                # stream delivers it; bf16 warm fillers plug the gaps.
                u_ps = pset.tile([1, 3, 512], f32, tag="u")
                for qi in range(4):
                    for ic in (2 * qi, 2 * qi + 1):
                        for jb in range(3):
                            nc.tensor.matmul(
                                u_ps[:, jb, :],
                                vb_sb[:, ic:ic + 1],
                                wchunk(ic, jb * 512, (jb + 1) * 512),
                                start=(ic == 0), stop=(ic == HC - 1))
                    warm(2)

                # bias = b @ v (8 rank-1s, ~1 cycle each)
                bias_ps = pset.tile([1, 1], f32, tag="bias")
                for ic in range(HC):
                    nc.tensor.matmul(
                        bias_ps, vb_sb[:, ic:ic + 1],
                        vb_sb[:, HC + ic:HC + ic + 1],
                        start=(ic == 0), stop=(ic == HC - 1))
                bias_sb = consts.tile([1, 1], f32, tag="bias_sb")
                nc.vector.tensor_copy(bias_sb, bias_ps)

                u_row = consts.tile([1, JC, 128], bf16, tag="urow")
                nc.vector.tensor_copy(
                    u_row.rearrange("p a b -> p (a b)").rearrange(
                        "p (x y) -> p x y", x=3), u_ps)

                # u columns (128, 12) via rank-1 transposes
                uc_ps = pset.tile([128, JC], f32, tag="uc")
                for jc in range(JC):
                    nc.tensor.matmul(
                        uc_ps[:, jc:jc + 1], u_row[0:1, jc, :],
                        ones[:, 0:1], start=True, stop=True)
                ucols = consts.tile([128, JC], bf16, tag="ucols")
                nc.vector.tensor_copy(ucols, uc_ps)

                # ev row (1, 256) then evb = ev + bias
                ev_ps = pset.tile([1, FW], f32, tag="ev")
                for kc in range(KC):
                    nc.tensor.matmul(
                        ev_ps, ucols[:, HC + kc:HC + kc + 1],
                        enc_sb[:, kc * FW:(kc + 1) * FW],
                        start=(kc == 0), stop=(kc == KC - 1))
                evb_row = consts.tile([1, FW], bf16, tag="evb")
                nc.vector.tensor_scalar_add(evb_row, ev_ps, bias_sb[:, 0:1])
                warm(4)

            # --- per n-block: contract over H, build score in PSUM, softmax
            o_all = consts.tile([128, NBLK * FW], bf16, tag="o_all")
            with tc.tile_pool(name="ps_main", bufs=1, space="PSUM") as pp:

                def finish(k, acc):
                    row = work.tile([1, BW], bf16, tag="row", bufs=3,
                                    name=f"row_{k}")
                    nc.vector.tensor_copy(row, acc)
                    rowv = row.rearrange("p (n bb) -> p bb n", bb=BPC)
                    sc_ps = pp.tile([128, FW], f32, tag="score", bufs=3,
                                    name=f"score_{k}")
                    nc.tensor.matmul(sc_ps, ones, evb_row,
                                     start=True, stop=False)
                    for bb in range(BPC):
                        nc.tensor.matmul(
                            sc_ps[:, bb * C:(bb + 1) * C],
                            rowv[0:1, bb, :], ones[:, 0:C],
                            start=False, stop=(bb == BPC - 1),
                            skip_group_check=True)
                    sc = work.tile([128, FW], f32, tag="sc", bufs=2,
                                   name=f"sc_{k}")
                    nc.scalar.activation(out=sc, in_=sc_ps, func=AF.Tanh)
                    nc.scalar.activation(out=sc, in_=sc, func=AF.Exp)
                    den = work.tile([128, BPC], f32, tag="den", bufs=2,
                                    name=f"den_{k}")
                    nc.vector.tensor_reduce(
                        den, sc.rearrange("p (bb c) -> p bb c", c=C),
                        axis=X, op=ADD)
                    nc.vector.reciprocal(den, den)
                    for bb in range(BPC):
                        nc.vector.tensor_scalar_mul(
                            o_all[:, k * FW + bb * C:k * FW + (bb + 1) * C],
                            sc[:, bb * C:(bb + 1) * C],
                            den[:, bb:bb + 1])

                for k in range(NBLK - 2):
                    acc = pp.tile([1, BW], f32, tag="acc", bufs=3,
                                  name=f"acc_{k}")
                    for hc in range(HC):
                        nc.tensor.matmul(
                            acc, ucols[:, hc:hc + 1],
                            hid_sb[k][:, hc * BW:(hc + 1) * BW],
                            start=(hc == 0), stop=(hc == HC - 1))
                    finish(k, acc)
                    if k == 3:
                        # first half of the output leaves mid-stream on the
                        # otherwise-idle SWDGE path
                        nc.gpsimd.dma_start(
                            out=out_d[:, 0:4 * FW], in_=o_all[:, 0:4 * FW])

                accs = {k: pp.tile([1, BW], f32, tag="acc", bufs=3,
                                   name=f"acc_{k}") for k in (6, 7)}
                for hf in range(2):
                    for k in (6, 7):
                        hw2 = BW // 2
                        for hc in range(HC):
                            nc.tensor.matmul(
                                accs[k][:, hf * hw2:(hf + 1) * hw2],
                                ucols[:, hc:hc + 1],
                                halves[k, hf][:, hc * hw2:(hc + 1) * hw2],
                                start=(hc == 0), stop=(hc == HC - 1),
                                skip_group_check=True)
                        if hf == 1:
                            finish(k, accs[k])
                nc.sync.dma_start(
                    out=out_d[:, 4 * FW:], in_=o_all[:, 4 * FW:])
    nc.compile()
    return nc


def kernel(**inputs):
    global _cached, LAST_RESULT
    hidden = np.asarray(inputs["hidden"], dtype=np.float32)
    enc = np.asarray(inputs["encoder_outputs"], dtype=np.float32)
    W = np.asarray(inputs["W"], dtype=np.float32)
    b = np.asarray(inputs["b"], dtype=np.float32)
    v = np.asarray(inputs["v"], dtype=np.float32)

    if _cached is None:
        _cached = _build()
    nc = _cached

    # vb: column ic holds v[ic*128:(ic+1)*128]; column HC+ic holds b chunk.
    vb = np.concatenate(
        [v.reshape(HC, 128).T, b.reshape(HC, 128).T], axis=1).astype(BF)
    # W partition-major quarters: wt[p, ic, j] = W[ic*128 + p, j]
    wt = W.astype(BF).reshape(HC, 128, WROW).transpose(1, 0, 2)
    q0 = np.ascontiguousarray(
        np.concatenate([vb, wt[:, 0:2].reshape(128, WQ)], axis=1))
    q1 = np.ascontiguousarray(wt[:, 2:4].reshape(128, WQ))
    q2 = np.ascontiguousarray(wt[:, 4:6].reshape(128, WQ))
    q3w = wt[:, 6:8].reshape(128, WQ)

    hb = hidden.astype(BF)
    eb = enc.astype(BF)

    in_maps = []
    for j in range(NCORES):
        bsl = slice(j * BPC, (j + 1) * BPC)
        # hid: [p, (k, hc, fn*BPC+bb)]; blocks 6/7 split into halves
        # [p, (k, half, hc, f)] so the stream tail arrives in 0.5MB steps.
        x = hb[:, bsl, :]                                   # (N, BPC, H)
        x = x.transpose(2, 0, 1)                            # (H, N, BPC)
        x = x.reshape(HC, 128, NBLK, BW)                    # (hc, p, k, f)
        full = x[:, :, :NBLK - 2].transpose(1, 2, 0, 3)     # (p, k, hc, f)
        tail = x[:, :, NBLK - 2:].reshape(HC, 128, 2, 2, BW // 2)
        tail = tail.transpose(1, 2, 3, 0, 4)                # (p,k,half,hc,f)
        hid_t = np.ascontiguousarray(np.concatenate(
            [full.reshape(128, -1), tail.reshape(128, -1)], axis=1))
        # enc: [p, kc*FW + bb*C + c]
        e = eb[:, bsl, :].transpose(2, 1, 0)                # (K, BPC, C)
        e = e.reshape(KC, 128, FW).transpose(1, 0, 2)
        enc_t = e.reshape(128, ENC_W)
        q3 = np.ascontiguousarray(np.concatenate([q3w, enc_t], axis=1))
        in_maps.append(
            {"hid": hid_t, "q0": q0, "q1": q1, "q2": q2, "q3": q3})

    res = run_bass_kernel_spmd(
        nc, in_maps, core_ids=list(range(NCORES)), trace=TRACE, **TRACE_KW)
    LAST_RESULT = res

    out = np.empty((B, N, C), dtype=np.float32)
    for j in range(NCORES):
        o = res.results[j]["out"].astype(np.float32)
        o = o.reshape(128, NBLK, BPC, C).transpose(2, 1, 0, 3)
        out[j * BPC:(j + 1) * BPC] = o.reshape(BPC, N, C)
    return out
